# revision 14
# baseline (speedup 1.0000x reference)
"""Causal multi-head attention on 8 trn2 NeuronCores (Megatron-style head parallelism).

Problem: B=2, L=2048, D=1024, H=16 heads (HD=64), fp32 in/out.

Sharding: each of the 8 cores owns 2 heads (a 128-wide slice of the QKV
projection output / Wo rows). Every core reads the full x; QKV projections are
column-sharded, attention runs per-head, the output projection is row-sharded
producing a partial sum per core which the host reduces (+ bo).

On-chip layout: activations are kept feature-major ("transposed"):
  x^T [D, B*L] (host pre-transposes), Q^T/K^T/V^T [128(d), L] per batch.
Scores are computed transposed: S^T[k, q] = K^T_blk.T @ Q^T (contraction over
head dim on partitions), softmax runs along partitions via an appended
ones-column in the V stationary operand (denominator lands in psum row 64),
and ctx^T[d, q] accumulates over key blocks with V-natural as lhsT.
Causality at 128-key-block granularity; diagonal blocks masked with
precomputed 0/1 tiles. exp needs no max-subtraction: |scores/8| < ~6 in fp32.

Matmul operands are bf16 (fp32 PSUM accumulation); the softmax denominator
reciprocal/broadcast path stays float32r so normalization keeps ~1e-4 error.
"""

import numpy as np

_B, _L, _D, _H, _HD = 2, 2048, 1024, 16, 64
_NC = 8
_DC = _D // _NC          # 128 feature dims (2 heads) per core
_T = _B * _L             # 4096 tokens
_NKB = _L // 128         # 16 key blocks per batch
_NQT = _L // 512         # 4 query tiles per batch

_cache = {}


def _build_bass():
    from concourse import bacc
    import concourse.mybir as mybir
    import concourse.tile as tile

    f32 = mybir.dt.float32
    f32r = mybir.dt.float32r
    bf16 = mybir.dt.bfloat16
    AFT = mybir.ActivationFunctionType

    nc = bacc.Bacc("TRN2", target_bir_lowering=False, debug=False, num_devices=_NC)

    xT = nc.dram_tensor("xT", [_D, _T], bf16, kind="ExternalInput")
    wq = nc.dram_tensor("wq", [_D, _DC], bf16, kind="ExternalInput")
    wk = nc.dram_tensor("wk", [_D, _DC], bf16, kind="ExternalInput")
    wv = nc.dram_tensor("wv", [_D, _DC], bf16, kind="ExternalInput")
    wo = nc.dram_tensor("wo", [_DC, _D], bf16, kind="ExternalInput")
    bqd = nc.dram_tensor("bq", [_DC, 1], f32, kind="ExternalInput")
    bkd = nc.dram_tensor("bk", [_DC, 1], f32, kind="ExternalInput")
    bvd = nc.dram_tensor("bv", [_DC, 1], f32, kind="ExternalInput")
    msk = nc.dram_tensor("msk", [4, 128, 512], bf16, kind="ExternalInput")
    idn = nc.dram_tensor("idn", [128, 128], bf16, kind="ExternalInput")
    ons = nc.dram_tensor("ons", [128, 65], f32r, kind="ExternalInput")
    onsb = nc.dram_tensor("onsb", [128, _NKB], bf16, kind="ExternalInput")
    out = nc.dram_tensor("out", [_T, _D], f32, kind="ExternalOutput")

    with tile.TileContext(nc) as tc:
        with (
            tc.tile_pool(name="const", bufs=1) as constp,
            tc.tile_pool(name="xt", bufs=10) as xtp,
            tc.tile_pool(name="qkv", bufs=2) as qkvp,
            tc.tile_pool(name="probs", bufs=6) as probsp,
            tc.tile_pool(name="stage", bufs=3) as stagep,
            tc.tile_pool(name="sc", bufs=2, space="PSUM") as scp,   # [128,1024] f32 = 2 banks each
            tc.tile_pool(name="cx", bufs=2, space="PSUM") as cxp,   # [128,512] f32 = 1 bank each
            tc.tile_pool(name="mi", bufs=2, space="PSUM") as mip,   # [128,512] slot = 1 bank each
        ):
            # ---- persistent constants ----
            wq_sb = constp.tile([128, 8, 128], bf16, tag="wq")
            wk_sb = constp.tile([128, 8, 128], bf16, tag="wk")
            wv_sb = constp.tile([128, 8, 128], bf16, tag="wv")
            nc.sync.dma_start(wq_sb[:], wq.rearrange("(c p) d -> p c d", p=128))
            nc.sync.dma_start(wk_sb[:], wk.rearrange("(c p) d -> p c d", p=128))
            nc.sync.dma_start(wv_sb[:], wv.rearrange("(c p) d -> p c d", p=128))
            wo0_sb = constp.tile([64, 1024], bf16, tag="wo0")
            wo1_sb = constp.tile([64, 1024], bf16, tag="wo1")
            nc.sync.dma_start(wo0_sb[:], wo[0:64, :])
            nc.sync.dma_start(wo1_sb[:], wo[64:128, :])
            bq_sb = constp.tile([128, 1], f32, tag="bq")
            bk_sb = constp.tile([128, 1], f32, tag="bk")
            bv_sb = constp.tile([128, 1], f32, tag="bv")
            nc.sync.dma_start(bq_sb[:], bqd[:])
            nc.sync.dma_start(bk_sb[:], bkd[:])
            nc.sync.dma_start(bv_sb[:], bvd[:])
            msk_sb = constp.tile([128, 4, 512], bf16, tag="msk")
            nc.sync.dma_start(msk_sb[:], msk.rearrange("i p q -> p i q"))
            idn_sb = constp.tile([128, 128], bf16, tag="idn")
            nc.sync.dma_start(idn_sb[:], idn[:])
            ons_sb = constp.tile([128, 65], f32r, tag="ons")
            nc.sync.dma_start(ons_sb[:], ons[:])
            onsb_sb = constp.tile([128, _NKB], bf16, tag="onsb")
            nc.sync.dma_start(onsb_sb[:], onsb[:])

            for b in range(_B):
                t0 = b * _L
                # ---- projections: Q^T, K^T, V^T [128(d), L] ----
                # x^T streams in per (1024-token strip, 128-dim chunk) so at
                # most 8 chunk tiles + lookahead are live (pool bufs=10).
                qT_sb = qkvp.tile([128, _L], bf16, tag="qT")
                kT_sb = qkvp.tile([128, _L], bf16, tag="kT")
                vT_sb = qkvp.tile([128, _L], bf16, tag="vT")
                for tb2 in range(_L // 1024):
                    xts = []
                    for ec in range(8):
                        xt_t = xtp.tile(
                            [128, 1024], bf16, tag="xt", name=f"xt{ec}"
                        )
                        nc.sync.dma_start(
                            xt_t[:],
                            xT[ec * 128:(ec + 1) * 128,
                               t0 + tb2 * 1024:t0 + (tb2 + 1) * 1024],
                        )
                        xts.append(xt_t)
                    for w_sb, b_sb, dst in (
                        (wq_sb, bq_sb, qT_sb),
                        (wk_sb, bk_sb, kT_sb),
                        (wv_sb, bv_sb, vT_sb),
                    ):
                        ps = scp.tile([128, 1024], f32, tag="sc")
                        for half in range(2):
                            col = half * 512
                            for ec in range(8):
                                nc.tensor.matmul(
                                    ps[:, col:col + 512],
                                    w_sb[:, ec, :],
                                    xts[ec][:, col:col + 512],
                                    start=(ec == 0),
                                    stop=(ec == 7),
                                )
                        nc.vector.tensor_scalar_add(
                            dst[:, tb2 * 1024:(tb2 + 1) * 1024], ps[:], b_sb[:]
                        )

                # ---- V natural: per key block, [tok, d] + ones column ----
                v0_sb = qkvp.tile([128, _NKB, 65], bf16, tag="v0")
                v1_sb = qkvp.tile([128, _NKB, 65], bf16, tag="v1")
                for kb in range(_NKB):
                    vt_ps = mip.tile([128, 512], bf16, tag="mi", name="vt_ps")
                    nc.tensor.transpose(
                        vt_ps[:, 0:128], vT_sb[:, kb * 128:(kb + 1) * 128], idn_sb[:]
                    )
                    nc.vector.tensor_copy(v0_sb[:, kb, 0:64], vt_ps[:, 0:64])
                    nc.vector.tensor_copy(v1_sb[:, kb, 0:64], vt_ps[:, 64:128])
                nc.vector.tensor_copy(v0_sb[:, :, 64], onsb_sb[:])
                nc.vector.tensor_copy(v1_sb[:, :, 64], onsb_sb[:])

                # ---- attention (2 heads packed on partition halves) ----
                ctx0_sb = qkvp.tile([64, _L], bf16, tag="ctx0")
                ctx1_sb = qkvp.tile([64, _L], bf16, tag="ctx1")
                for qt in range(_NQT):
                    nk = 4 * (qt + 1)       # causal: key blocks 0..nk-1
                    q0 = qt * 512
                    ctx_ps = [
                        cxp.tile([128, 512], f32, tag="cx", name=f"ctx_ps{h}")
                        for h in range(2)
                    ]
                    for kb in range(nk):
                        sc_ps = scp.tile([128, 1024], f32, tag="sc")
                        for h in range(2):
                            hp = h * 64
                            nc.tensor.matmul(
                                sc_ps[:, h * 512:(h + 1) * 512],
                                kT_sb[hp:hp + 64, kb * 128:(kb + 1) * 128],
                                qT_sb[hp:hp + 64, q0:q0 + 512],
                                start=True, stop=True,
                            )
                        pr = probsp.tile([128, 1024], bf16, tag="pr")
                        nc.scalar.activation(pr[:], sc_ps[:], AFT.Exp, scale=0.125)
                        if kb >= nk - 4:
                            mi_idx = kb - (nk - 4)
                            for h in range(2):
                                nc.vector.tensor_mul(
                                    pr[:, h * 512:(h + 1) * 512],
                                    pr[:, h * 512:(h + 1) * 512],
                                    msk_sb[:, mi_idx, :],
                                )
                        for h, v_sb in ((0, v0_sb), (1, v1_sb)):
                            nc.tensor.matmul(
                                ctx_ps[h][0:65, :],
                                v_sb[:, kb, :],
                                pr[:, h * 512:(h + 1) * 512],
                                start=(kb == 0), stop=(kb == nk - 1),
                            )
                    for h in range(2):
                        ctx_sb = ctx0_sb if h == 0 else ctx1_sb
                        rc = stagep.tile([128, 512], f32r, tag="rc")
                        with nc.allow_low_precision(
                            reason="f32r reciprocal feeds f32r matmul; ~1e-3 ok"
                        ):
                            nc.vector.reciprocal(rc[64:65, :], ctx_ps[h][64:65, :])
                        bc_ps = mip.tile([128, 512], f32, tag="mi")
                        nc.tensor.matmul(
                            bc_ps[0:65, :], ons_sb[64:65, :], rc[64:65, :],
                            start=True, stop=True,
                        )
                        bc_sb = stagep.tile([64, 512], f32, tag="bc")
                        nc.vector.tensor_copy(bc_sb[:], bc_ps[0:64, :])
                        nc.vector.tensor_mul(
                            ctx_sb[0:64, q0:q0 + 512],
                            ctx_ps[h][0:64, :],
                            bc_sb[:],
                        )

                    # ---- output projection for this q-tile's 4 token blocks ----
                    for tkb in range(qt * 4, qt * 4 + 4):
                        stg = stagep.tile([128, 1024], f32, tag="og")
                        for nch in range(2):
                            op_ps = mip.tile([128, 512], f32, tag="mi", name="op_ps")
                            nc.tensor.matmul(
                                op_ps[:], ctx0_sb[0:64, tkb * 128:(tkb + 1) * 128],
                                wo0_sb[:, nch * 512:(nch + 1) * 512],
                                start=True, stop=False,
                            )
                            nc.tensor.matmul(
                                op_ps[:], ctx1_sb[0:64, tkb * 128:(tkb + 1) * 128],
                                wo1_sb[:, nch * 512:(nch + 1) * 512],
                                start=False, stop=True,
                            )
                            nc.vector.tensor_copy(
                                stg[:, nch * 512:(nch + 1) * 512], op_ps[:]
                            )
                        r0 = t0 + tkb * 128
                        nc.sync.dma_start(out[r0:r0 + 128, :], stg[:])

    nc.compile()
    return nc


def _get_nc():
    if "nc" not in _cache:
        _cache["nc"] = _build_bass()
    return _cache["nc"]


def _host_inputs(x, Wq, bq, Wk, bk, Wv, bv, Wo, bo):
    import ml_dtypes

    bf = ml_dtypes.bfloat16
    x = np.asarray(x, np.float32)
    xT = np.ascontiguousarray(x.reshape(_T, _D).T.astype(bf))

    # diagonal-block causal masks: mask[i][k, q] = 1 if (128*i + k) <= q
    kk = np.arange(128)[:, None]
    qq = np.arange(512)[None, :]
    masks = np.stack(
        [(qq >= kk + 128 * i).astype(bf) for i in range(4)]
    )
    ident = np.eye(128, dtype=bf)
    ones = np.ones((128, 65), np.float32)
    onesb = np.ones((128, _NKB), bf)

    in_maps = []
    for c in range(_NC):
        s = slice(c * _DC, (c + 1) * _DC)
        in_maps.append({
            "xT": xT,
            "wq": np.ascontiguousarray(np.asarray(Wq, np.float32)[:, s].astype(bf)),
            "wk": np.ascontiguousarray(np.asarray(Wk, np.float32)[:, s].astype(bf)),
            "wv": np.ascontiguousarray(np.asarray(Wv, np.float32)[:, s].astype(bf)),
            "wo": np.ascontiguousarray(np.asarray(Wo, np.float32)[s, :].astype(bf)),
            "bq": np.ascontiguousarray(np.asarray(bq, np.float32)[s, None]),
            "bk": np.ascontiguousarray(np.asarray(bk, np.float32)[s, None]),
            "bv": np.ascontiguousarray(np.asarray(bv, np.float32)[s, None]),
            "msk": masks,
            "idn": ident,
            "ons": ones,
            "onsb": onesb,
        })
    return in_maps


def kernel_run(x, Wq, bq, Wk, bk, Wv, bv, Wo, bo, trace=False):
    """Run the SPMD kernel; returns (full output, BassKernelResults)."""
    from concourse.bass_utils import run_bass_kernel_spmd

    nc = _get_nc()
    in_maps = _host_inputs(x, Wq, bq, Wk, bk, Wv, bv, Wo, bo)
    res = run_bass_kernel_spmd(nc, in_maps, list(range(_NC)), trace=trace)
    acc = np.zeros((_T, _D), np.float32)
    for c in range(_NC):
        acc += res.results[c]["out"]
    acc += np.asarray(bo, np.float32)[None, :]
    return acc.reshape(_B, _L, _D), res


def kernel(x, Wq, bq, Wk, bk, Wv, bv, Wo, bo):
    out, _ = kernel_run(x, Wq, bq, Wk, bk, Wv, bv, Wo, bo, trace=False)
    return out


# revision 18
# speedup vs baseline: 1.1693x; 1.1693x over previous
"""Causal multi-head attention on 8 trn2 NeuronCores (Megatron-style head parallelism).

Problem: B=2, L=2048, D=1024, H=16 heads (HD=64), fp32 in/out.

Sharding: each of the 8 cores owns 2 heads (a 128-wide slice of the QKV
projection output / Wo rows). Every core reads the full x; QKV projections are
column-sharded, attention runs per-head, the output projection is row-sharded
producing a partial sum per core which the host reduces (+ bo).

On-chip layout: activations are kept feature-major ("transposed"):
  x^T [D, B*L] (host pre-transposes), Q^T/K^T/V^T [128(d), L] per batch.
Scores are computed transposed: S^T[k, q] = K^T_blk.T @ Q^T (contraction over
head dim on partitions), softmax runs along partitions via an appended
ones-column in the V stationary operand (denominator lands in psum row 64),
and ctx^T[d, q] accumulates over key blocks with V-natural as lhsT.
Causality at 128-key-block granularity; diagonal blocks masked with
precomputed 0/1 tiles. exp needs no max-subtraction: |scores/8| < ~6 in fp32.

Matmul operands are bf16 (fp32 PSUM accumulation); the softmax denominator
reciprocal/broadcast path stays float32r so normalization keeps ~1e-4 error.
"""

import numpy as np

_B, _L, _D, _H, _HD = 2, 2048, 1024, 16, 64
_NC = 8
_DC = _D // _NC          # 128 feature dims (2 heads) per core
_T = _B * _L             # 4096 tokens
_NKB = _L // 128         # 16 key blocks per batch
_NQT = _L // 512         # 4 query tiles per batch

_cache = {}


def _build_bass():
    from concourse import bacc
    import concourse.mybir as mybir
    import concourse.tile as tile

    f32 = mybir.dt.float32
    f32r = mybir.dt.float32r
    bf16 = mybir.dt.bfloat16
    AFT = mybir.ActivationFunctionType

    nc = bacc.Bacc("TRN2", target_bir_lowering=False, debug=False, num_devices=_NC)

    xT = nc.dram_tensor("xT", [_D, _T], bf16, kind="ExternalInput")
    wq = nc.dram_tensor("wq", [_D, _DC], bf16, kind="ExternalInput")
    wk = nc.dram_tensor("wk", [_D, _DC], bf16, kind="ExternalInput")
    wv = nc.dram_tensor("wv", [_D, _DC], bf16, kind="ExternalInput")
    wo = nc.dram_tensor("wo", [_DC, _D], bf16, kind="ExternalInput")
    bqd = nc.dram_tensor("bq", [_DC, 1], f32, kind="ExternalInput")
    bkd = nc.dram_tensor("bk", [_DC, 1], f32, kind="ExternalInput")
    bvd = nc.dram_tensor("bv", [_DC, 1], f32, kind="ExternalInput")
    msk = nc.dram_tensor("msk", [4, 128, 512], bf16, kind="ExternalInput")
    idn = nc.dram_tensor("idn", [128, 128], bf16, kind="ExternalInput")
    ons = nc.dram_tensor("ons", [128, 65], f32r, kind="ExternalInput")
    onsb = nc.dram_tensor("onsb", [128, _NKB], bf16, kind="ExternalInput")
    out = nc.dram_tensor("out", [_T, _D], f32, kind="ExternalOutput")

    with tile.TileContext(nc) as tc:
        with (
            tc.tile_pool(name="const", bufs=1) as constp,
            tc.tile_pool(name="xt", bufs=10) as xtp,
            tc.tile_pool(name="qkv", bufs=2) as qkvp,
            tc.tile_pool(name="probs", bufs=4) as probsp,
            tc.tile_pool(name="stage", bufs=3) as stagep,
            tc.tile_pool(name="sc", bufs=2, space="PSUM") as scp,   # [128,1024] f32 = 2 banks each
            tc.tile_pool(name="cx", bufs=2, space="PSUM") as cxp,   # [128,512] f32 = 1 bank each
            tc.tile_pool(name="mi", bufs=2, space="PSUM") as mip,   # [128,512] slot = 1 bank each
        ):
            # ---- persistent constants ----
            wq_sb = constp.tile([128, 8, 128], bf16, tag="wq")
            wk_sb = constp.tile([128, 8, 128], bf16, tag="wk")
            wv_sb = constp.tile([128, 8, 128], bf16, tag="wv")
            nc.sync.dma_start(wq_sb[:], wq.rearrange("(c p) d -> p c d", p=128))
            nc.sync.dma_start(wk_sb[:], wk.rearrange("(c p) d -> p c d", p=128))
            nc.sync.dma_start(wv_sb[:], wv.rearrange("(c p) d -> p c d", p=128))
            wo0_sb = constp.tile([64, 1024], bf16, tag="wo0")
            wo1_sb = constp.tile([64, 1024], bf16, tag="wo1")
            nc.sync.dma_start(wo0_sb[:], wo[0:64, :])
            nc.sync.dma_start(wo1_sb[:], wo[64:128, :])
            bq_sb = constp.tile([128, 1], f32, tag="bq")
            bk_sb = constp.tile([128, 1], f32, tag="bk")
            bv_sb = constp.tile([128, 1], f32, tag="bv")
            nc.sync.dma_start(bq_sb[:], bqd[:])
            nc.sync.dma_start(bk_sb[:], bkd[:])
            nc.sync.dma_start(bv_sb[:], bvd[:])
            msk_sb = constp.tile([128, 4, 512], bf16, tag="msk")
            nc.sync.dma_start(msk_sb[:], msk.rearrange("i p q -> p i q"))
            idn_sb = constp.tile([128, 128], bf16, tag="idn")
            nc.sync.dma_start(idn_sb[:], idn[:])
            ons_sb = constp.tile([128, 65], f32r, tag="ons")
            nc.sync.dma_start(ons_sb[:], ons[:])
            onsb_sb = constp.tile([128, _NKB], bf16, tag="onsb")
            nc.sync.dma_start(onsb_sb[:], onsb[:])

            for b in range(_B):
                t0 = b * _L
                # ---- projections: Q^T, K^T, V^T [128(d), L] ----
                # x^T streams in per (1024-token strip, 128-dim chunk) so at
                # most 8 chunk tiles + lookahead are live (pool bufs=10).
                qT_sb = qkvp.tile([128, _L], bf16, tag="qT")
                kT_sb = qkvp.tile([128, _L], bf16, tag="kT")
                vT_sb = qkvp.tile([128, _L], bf16, tag="vT", bufs=1)
                for tb2 in range(_L // 1024):
                    xts = []
                    for ec in range(8):
                        xt_t = xtp.tile(
                            [128, 1024], bf16, tag="xt", name=f"xt{ec}"
                        )
                        nc.sync.dma_start(
                            xt_t[:],
                            xT[ec * 128:(ec + 1) * 128,
                               t0 + tb2 * 1024:t0 + (tb2 + 1) * 1024],
                        )
                        xts.append(xt_t)
                    for w_sb, b_sb, dst in (
                        (wq_sb, bq_sb, qT_sb),
                        (wk_sb, bk_sb, kT_sb),
                        (wv_sb, bv_sb, vT_sb),
                    ):
                        ps = scp.tile([128, 1024], f32, tag="sc")
                        for half in range(2):
                            col = half * 512
                            for ec in range(8):
                                nc.tensor.matmul(
                                    ps[:, col:col + 512],
                                    w_sb[:, ec, :],
                                    xts[ec][:, col:col + 512],
                                    start=(ec == 0),
                                    stop=(ec == 7),
                                )
                        nc.vector.tensor_scalar_add(
                            dst[:, tb2 * 1024:(tb2 + 1) * 1024], ps[:], b_sb[:]
                        )

                # ---- V natural: per key block, [tok, d] + ones column ----
                v0_sb = qkvp.tile([128, _NKB, 65], bf16, tag="v0")
                v1_sb = qkvp.tile([128, _NKB, 65], bf16, tag="v1")
                for kb in range(_NKB):
                    vt_ps = mip.tile([128, 512], bf16, tag="mi", name="vt_ps")
                    nc.tensor.transpose(
                        vt_ps[:, 0:128], vT_sb[:, kb * 128:(kb + 1) * 128], idn_sb[:]
                    )
                    nc.vector.tensor_copy(v0_sb[:, kb, 0:64], vt_ps[:, 0:64])
                    nc.vector.tensor_copy(v1_sb[:, kb, 0:64], vt_ps[:, 64:128])
                nc.vector.tensor_copy(v0_sb[:, :, 64], onsb_sb[:])
                nc.vector.tensor_copy(v1_sb[:, :, 64], onsb_sb[:])

                # ---- attention (2 heads packed on partition halves) ----
                ctx0_sb = qkvp.tile([64, _L], bf16, tag="ctx0")
                ctx1_sb = qkvp.tile([64, _L], bf16, tag="ctx1")
                for qt in range(_NQT):
                    nk = 4 * (qt + 1)       # causal: key blocks 0..nk-1
                    q0 = qt * 512
                    ctx_ps = [
                        cxp.tile([128, 512], f32, tag="cx", name=f"ctx_ps{h}")
                        for h in range(2)
                    ]
                    for kb in range(nk):
                        sc_ps = scp.tile([128, 1024], f32, tag="sc")
                        for h in range(2):
                            hp = h * 64
                            nc.tensor.matmul(
                                sc_ps[:, h * 512:(h + 1) * 512],
                                kT_sb[hp:hp + 64, kb * 128:(kb + 1) * 128],
                                qT_sb[hp:hp + 64, q0:q0 + 512],
                                start=True, stop=True,
                            )
                        pr = probsp.tile([128, 1024], bf16, tag="pr")
                        nc.scalar.activation(pr[:], sc_ps[:], AFT.Exp, scale=0.125)
                        if kb >= nk - 4:
                            # diagonal block: cols < off are fully masked
                            # (gpsimd memset), the 128-wide strip at [off,
                            # off+128) needs the triangular 0/1 multiply.
                            off = (kb - (nk - 4)) * 128
                            for h in range(2):
                                c0 = h * 512
                                if off > 0:
                                    nc.vector.memset(pr[:, c0:c0 + off], 0.0)
                                nc.vector.tensor_mul(
                                    pr[:, c0 + off:c0 + off + 128],
                                    pr[:, c0 + off:c0 + off + 128],
                                    msk_sb[:, 0, 0:128],
                                )
                        for h, v_sb in ((0, v0_sb), (1, v1_sb)):
                            nc.tensor.matmul(
                                ctx_ps[h][0:65, :],
                                v_sb[:, kb, :],
                                pr[:, h * 512:(h + 1) * 512],
                                start=(kb == 0), stop=(kb == nk - 1),
                            )
                    for h in range(2):
                        ctx_sb = ctx0_sb if h == 0 else ctx1_sb
                        rc = stagep.tile([128, 512], f32r, tag="rc")
                        with nc.allow_low_precision(
                            reason="f32r reciprocal feeds f32r matmul; ~1e-3 ok"
                        ):
                            nc.vector.reciprocal(rc[64:65, :], ctx_ps[h][64:65, :])
                        bc_ps = mip.tile([128, 512], f32, tag="mi")
                        nc.tensor.matmul(
                            bc_ps[0:65, :], ons_sb[64:65, :], rc[64:65, :],
                            start=True, stop=True,
                        )
                        bc_sb = stagep.tile([64, 512], f32, tag="bc")
                        nc.vector.tensor_copy(bc_sb[:], bc_ps[0:64, :])
                        nc.vector.tensor_mul(
                            ctx_sb[0:64, q0:q0 + 512],
                            ctx_ps[h][0:64, :],
                            bc_sb[:],
                        )

                # ---- output projection (partial sums over this core's 128 dims) ----
                for tkb in range(_NKB):
                    stg = stagep.tile([128, 1024], f32, tag="og")
                    for nch in range(2):
                        op_ps = mip.tile([128, 512], f32, tag="mi", name="op_ps")
                        nc.tensor.matmul(
                            op_ps[:], ctx0_sb[0:64, tkb * 128:(tkb + 1) * 128],
                            wo0_sb[:, nch * 512:(nch + 1) * 512],
                            start=True, stop=False,
                        )
                        nc.tensor.matmul(
                            op_ps[:], ctx1_sb[0:64, tkb * 128:(tkb + 1) * 128],
                            wo1_sb[:, nch * 512:(nch + 1) * 512],
                            start=False, stop=True,
                        )
                        nc.vector.tensor_copy(stg[:, nch * 512:(nch + 1) * 512], op_ps[:])
                    r0 = t0 + tkb * 128
                    nc.sync.dma_start(out[r0:r0 + 128, :], stg[:])

    nc.compile()
    return nc


def _get_nc():
    if "nc" not in _cache:
        _cache["nc"] = _build_bass()
    return _cache["nc"]


def _host_inputs(x, Wq, bq, Wk, bk, Wv, bv, Wo, bo):
    import ml_dtypes

    bf = ml_dtypes.bfloat16
    x = np.asarray(x, np.float32)
    xT = np.ascontiguousarray(x.reshape(_T, _D).T.astype(bf))

    # diagonal-block causal masks: mask[i][k, q] = 1 if (128*i + k) <= q
    kk = np.arange(128)[:, None]
    qq = np.arange(512)[None, :]
    masks = np.stack(
        [(qq >= kk + 128 * i).astype(bf) for i in range(4)]
    )
    ident = np.eye(128, dtype=bf)
    ones = np.ones((128, 65), np.float32)
    onesb = np.ones((128, _NKB), bf)

    in_maps = []
    for c in range(_NC):
        s = slice(c * _DC, (c + 1) * _DC)
        in_maps.append({
            "xT": xT,
            "wq": np.ascontiguousarray(np.asarray(Wq, np.float32)[:, s].astype(bf)),
            "wk": np.ascontiguousarray(np.asarray(Wk, np.float32)[:, s].astype(bf)),
            "wv": np.ascontiguousarray(np.asarray(Wv, np.float32)[:, s].astype(bf)),
            "wo": np.ascontiguousarray(np.asarray(Wo, np.float32)[s, :].astype(bf)),
            "bq": np.ascontiguousarray(np.asarray(bq, np.float32)[s, None]),
            "bk": np.ascontiguousarray(np.asarray(bk, np.float32)[s, None]),
            "bv": np.ascontiguousarray(np.asarray(bv, np.float32)[s, None]),
            "msk": masks,
            "idn": ident,
            "ons": ones,
            "onsb": onesb,
        })
    return in_maps


def kernel_run(x, Wq, bq, Wk, bk, Wv, bv, Wo, bo, trace=False):
    """Run the SPMD kernel; returns (full output, BassKernelResults)."""
    from concourse.bass_utils import run_bass_kernel_spmd

    nc = _get_nc()
    in_maps = _host_inputs(x, Wq, bq, Wk, bk, Wv, bv, Wo, bo)
    res = run_bass_kernel_spmd(nc, in_maps, list(range(_NC)), trace=trace)
    acc = np.zeros((_T, _D), np.float32)
    for c in range(_NC):
        acc += res.results[c]["out"]
    acc += np.asarray(bo, np.float32)[None, :]
    return acc.reshape(_B, _L, _D), res


def kernel(x, Wq, bq, Wk, bk, Wv, bv, Wo, bo):
    out, _ = kernel_run(x, Wq, bq, Wk, bk, Wv, bv, Wo, bo, trace=False)
    return out
